# revision 57
# baseline (speedup 1.0000x reference)
"""MiniFastSpeech Trainium2 kernel.

Strategy:
- Host (numpy): embed lookup, duration predictor, cumsum, searchsorted
  length-regulator expansion -> exp [B, L, E]; pad to L_PAD = 32*CHUNK.
- Device (8 cores, SPMD): bidirectional LSTM via sequence-chunked
  parallelism. LSTM state sensitivity decays exponentially (product of
  forget gates), so each chunk runs W=12 warmup steps from zero state
  before its real range (~5e-3 rel error vs the 2e-2 tolerance).
- 32 chunks per direction, CHUNK=22, L_PAD=704. Core j runs FOUR
  pair-chains (each fuses 2 chunks of one direction on the 128-wide
  partition dim = batch 64 x 2 chunks):
    F0: fwd chunks (4j, 4j+1)      F1: fwd chunks (4j+2, 4j+3)
    B0: bwd chunks (31-4j, 30-4j)  B1: bwd chunks (29-4j, 28-4j)
  over the REVERSED sequence; B-pairs cover the same real positions as
  the F-pairs, so the final linear is core-local. Four independent
  recurrences hide the ~5us per-step dependency-chain latency.
- Flat slot-stream schedule (one chain-slot at a time) with lagged
  emission tuned against the TimelineSim cost model: per slot t we emit
  xe projections for slots[t-3] (opens the PSUM group right before its
  accumulation; the 3-deep gates ring then has no cross-engine WAR),
  recurrent matmuls for slots[t+1], this chain's sigmoid stage,
  tanh(c)+h-mul for slots[t-2] (deps a full slot old -> no ACT stall),
  then ig/add, then the PE transpose + copy for slots[t-2].
- ALL-SIGMOID gates: host scales the g-gate weight rows by -2 so
  tanh(g) = 1 - 2*sigmoid(-2g); one 1024-wide sigmoid covers every
  gate and i*tanh(g) folds into a fused scalar_tensor_tensor. Host
  gate order [i,f,g,o] -> [g,f,i,o]. ACT per chain-step is just
  sigmoid(1024) + tanh(c): the binding engine at ~85% in-loop.
- fp16 storage for weights, the xe stream, h, and the X accumulator
  (so 4 chains fit SBUF); gates/c stay fp32 in PSUM/SBUF. fp16
  matmuls run 1 cycle/row like f32r; fp16 transposes 1 cycle/row.
- A ~24-matmul identity-transpose pre-warmer burns the PE p-state
  ramp (0.65 -> 2.4 GHz over 3us) while the first DMAs stream; DMA
  order: xe step-0 cols, wih, whh, rest. fp16 output staging with
  split DMAs overlapping the final-linear groups.
- Pointwise split: fc+add on Pool, t1/ig/h-mul/copy on DVE.
"""

import sys
import numpy as np
from contextlib import ExitStack

sys.path.insert(0, "/opt/trn_rl_repo")

import concourse.bass as bass
import concourse.tile as tile
from concourse import bacc, mybir
from concourse.bass_utils import run_bass_kernel_spmd
from concourse.masks import make_identity

# ---- problem constants (hardcoded per contract) ----
VOCAB, EMB, HID, MEL = 256, 128, 256, 80
B, T = 64, 512
N_CORES = 8
NCHUNK = 32          # chunks per direction
W = 12               # warmup steps per chain
CHUNK = 22           # positions per chunk; L_PAD = 704 >= L
L_PAD = NCHUNK * CHUNK
K_STEPS = W + CHUNK  # 34
NCH = 4              # pair-chains per core
POS_CORE = NCH * CHUNK  # 88 positions per core
XBLK = 8             # steps per bulk xe DMA block
NBLK = (K_STEPS + XBLK - 1) // XBLK
G4 = 4 * HID         # 1024
F32 = mybir.dt.float32
F32R = mybir.dt.float32r
F16 = mybir.dt.float16
SIG = mybir.ActivationFunctionType.Sigmoid
TANH = mybir.ActivationFunctionType.Tanh
IDENT = mybir.ActivationFunctionType.Identity

_COMPILED = None


def _host_expand(x, embed, dp_w, dp_b):
    xe = embed[x]                                   # (B,T,E)
    d = np.maximum(xe @ dp_w[0] + dp_b[0], 0)
    dur = np.floor(d).astype(np.int64) + 1
    cum = np.cumsum(dur, axis=1)
    L = int(cum[:, -1].max())
    pos = np.arange(L)
    idx = np.empty((B, L), np.int64)
    for b in range(B):
        idx[b] = np.searchsorted(cum[b], pos, side="right")
    mask = (pos[None, :] < cum[:, -1:]).astype(np.float32)
    exp = np.take_along_axis(xe, np.clip(idx, 0, T - 1)[..., None], axis=1)
    return np.ascontiguousarray(exp * mask[..., None], dtype=np.float32), L


def _gate_perm():
    i = np.arange(HID)
    # PyTorch order [i, f, g, o] -> device order [g, f, i, o]
    return np.concatenate([2 * HID + i, HID + i, i, 3 * HID + i])


class _Chain:
    """One fused pair-chain (two chunks of one direction)."""

    def __init__(self, name, dirn, wih, whh, xe_cols, slot0):
        self.name = name
        self.dirn = dirn
        self.wih = wih
        self.whh = whh
        self.xe_cols = xe_cols
        self.slot0 = slot0        # X slot index (hid-half k0) of this chain
        self.gates = None
        self.gates_next = None
        self.src0 = None
        self.src1 = None
        self.c_prev = None
        self.sfio = None
        self.t1 = None
        self.fc = None
        self.h = None


def _build_kernel():
    nc = bacc.Bacc("TRN2", target_bir_lowering=False, debug=False,
                   num_devices=N_CORES)

    # xein partition-major fp16: [EMB, s*512 + c]; per-step cols c:
    # chain ci in (F0,F1,B0,B1) at [ci*128:(ci+1)*128], chunk-a 64|chunk-b 64
    xein = nc.dram_tensor("xein", [EMB, NBLK * XBLK * 512], F16,
                          kind="ExternalInput").ap()
    wih_f_d = nc.dram_tensor("wihT_f", [1, EMB, G4], F16, kind="ExternalInput").ap()
    wih_b_d = nc.dram_tensor("wihT_b", [1, EMB, G4], F16, kind="ExternalInput").ap()
    whh_f_d = nc.dram_tensor("whhT_f", [128, 2 * G4], F16, kind="ExternalInput").ap()
    whh_b_d = nc.dram_tensor("whhT_b", [128, 2 * G4], F16, kind="ExternalInput").ap()
    lin_w_d = nc.dram_tensor("linT", [128, 4 * MEL], F16, kind="ExternalInput").ap()
    lin_b_d = nc.dram_tensor("lin_b", [MEL, 1], F32, kind="ExternalInput").ap()
    zeros_d = nc.dram_tensor("zeros", [128, 256], F16, kind="ExternalInput").ap()
    out_d = nc.dram_tensor("out_mel", [MEL, POS_CORE, B], F16,
                           kind="ExternalOutput").ap()

    with tile.TileContext(nc) as tc, ExitStack() as ctx:
        wpool = ctx.enter_context(tc.tile_pool(name="weights", bufs=1))
        xpool = ctx.enter_context(tc.tile_pool(name="xstream", bufs=2))
        state = ctx.enter_context(tc.tile_pool(name="state", bufs=2))
        actp = ctx.enter_context(tc.tile_pool(name="acts", bufs=6))
        xbig = ctx.enter_context(tc.tile_pool(name="xbig", bufs=1))
        scr = ctx.enter_context(tc.tile_pool(name="scratch", bufs=6))
        gpsum = ctx.enter_context(tc.tile_pool(name="gates", bufs=3, space="PSUM"))
        tpsum = ctx.enter_context(tc.tile_pool(name="trans", bufs=2, space="PSUM"))
        ostage = ctx.enter_context(tc.tile_pool(name="ostage", bufs=1))

        # ---- bulk xe streaming pool (block 0 DMA emitted FIRST so the
        # priming matmuls aren't stuck behind the weight transfers) ----
        xe_blocks = {}

        def load_block(b):
            if b in xe_blocks or b >= NBLK:
                return
            t = xpool.tile([EMB, XBLK * 512], F16, tag="xeblk",
                           name=f"xeblk{b}")
            nc.sync.dma_start(t[:],
                              xein[:, b * XBLK * 512:(b + 1) * XBLK * 512])
            xe_blocks[b] = t

        # block 0 split: step-0 columns + input weights land first so
        # priming starts ~4us in, right as the PE pre-warmer finishes
        blk0 = xpool.tile([EMB, XBLK * 512], F16, tag="xeblk", name="xeblk0")
        nc.sync.dma_start(blk0[:, 0:512], xein[:, 0:512])
        xe_blocks[0] = blk0

        # ---- weights -> SBUF (fp16) ----
        wih_f = wpool.tile([EMB, G4], F16, tag="wihf")
        nc.sync.dma_start(wih_f[:], wih_f_d[0])
        wih_b = wpool.tile([EMB, G4], F16, tag="wihb")
        nc.sync.dma_start(wih_b[:], wih_b_d[0])
        whh_f = wpool.tile([128, 2 * G4], F16, tag="whhf")
        nc.sync.dma_start(whh_f[:], whh_f_d[:])
        whh_b = wpool.tile([128, 2 * G4], F16, tag="whhb")
        nc.sync.dma_start(whh_b[:], whh_b_d[:])
        hT0 = wpool.tile([128, 256], F16, tag="hT0")
        nc.sync.dma_start(hT0[:], zeros_d[:])
        nc.sync.dma_start(blk0[:, 512:], xein[:, 512:XBLK * 512])
        load_block(1)
        lin_w = wpool.tile([128, 4 * MEL], F16, tag="linw")
        nc.sync.dma_start(lin_w[:], lin_w_d[:])
        lin_b = wpool.tile([MEL, 1], F32, tag="linb")
        nc.sync.dma_start(lin_b[:], lin_b_d[:])
        ident = wpool.tile([128, 128], F16, tag="ident")
        make_identity(nc, ident[:])
        warm = tpsum.tile([128, 256], F16, tag="ht", name="warmup")
        for wj in range(24):
            nc.tensor.matmul(warm[:, 0:128], ident[:], ident[:],
                             start=True, stop=True, is_transpose=True)

        # ---- X accumulator (fp16): one tile, 8 slots of [CHUNK lp x 128].
        # slot order: F0k0 F0k1 F1k0 F1k1 B0k0 B0k1 B1k0 B1k1; within a
        # slot, col = lp*128 + (chunk a|b)*64 + batch.
        X = xbig.tile([128, 8 * CHUNK * 128], F16, tag="X", name="X")
        X4 = X[:].rearrange("p (q l c) -> p q l c", q=8, l=CHUNK)

        chains = [
            _Chain("f0", "f", wih_f, whh_f, slice(0, 128), 0),
            _Chain("f1", "f", wih_f, whh_f, slice(128, 256), 2),
            _Chain("b0", "b", wih_b, whh_b, slice(256, 384), 4),
            _Chain("b1", "b", wih_b, whh_b, slice(384, 512), 6),
        ]
        for ch in chains:
            ch.src0 = hT0[:, 0:128]
            ch.src1 = hT0[:, 128:256]
            c0 = state.tile([128, HID], F32, tag="c" + ch.name,
                            name=f"c0{ch.name}")
            nc.gpsimd.memset(c0[:], 0.0)
            ch.c_prev = c0

        def emit_xe_mms(ch, s):
            b, off = divmod(s, XBLK)
            xe = xe_blocks[b][:, off * 512:(off + 1) * 512]
            g = gpsum.tile([128, G4], F32, tag="g", name=f"g{ch.name}{s}")
            for bank in (0, 1):
                nsl = slice(bank * 512, bank * 512 + 512)
                nc.tensor.matmul(g[:, nsl], xe[:, ch.xe_cols], ch.wih[:, nsl],
                                 start=True, stop=False)
            return g

        def emit_rec_mms(ch, s):
            for bank in (0, 1):
                nsl = slice(bank * 512, bank * 512 + 512)
                nc.tensor.matmul(ch.gates[:, nsl], ch.src0,
                                 ch.whh[:, bank * 512:bank * 512 + 512],
                                 start=False, stop=False)
                nc.tensor.matmul(ch.gates[:, nsl], ch.src1,
                                 ch.whh[:, G4 + bank * 512:G4 + bank * 512 + 512],
                                 start=False, stop=True)

        def emit_pw_sig_a(ch, s):
            """cols: [0:256]=g [256:512]=f [512:768]=i [768:1024]=o.
            g-rows of the weights are host-scaled by -2 so tanh(g) =
            1 - 2*sigmoid(-2g): ONE 1024-wide sigmoid covers all gates;
            i*tanh(g) = sgi - 2*(sgi*sgg) via a fused scalar_tensor_tensor."""
            nm = f"{ch.name}{s}"
            gates = ch.gates
            sall = actp.tile([128, G4], F32, tag="sfio", name="sf" + nm)
            nc.scalar.activation(sall[:], gates[:, 0:G4], SIG)
            t1 = scr.tile([128, HID], F32, tag="t1", name="t1" + nm)
            nc.vector.tensor_mul(t1[:], sall[:, 512:768], sall[:, 0:256])
            fc = scr.tile([128, HID], F32, tag="fc", name="fc" + nm)
            nc.gpsimd.tensor_mul(fc[:], sall[:, 256:512], ch.c_prev[:])
            ch.sfio = sall
            ch.t1 = t1
            ch.fc = fc

        def emit_pw_sig_b(ch, s):
            nm = f"{ch.name}{s}"
            sall = ch.sfio
            ig = scr.tile([128, HID], F32, tag="ig", name="ig" + nm)
            nc.vector.scalar_tensor_tensor(ig[:], ch.t1[:], -2.0,
                                           sall[:, 512:768],
                                           mybir.AluOpType.mult,
                                           mybir.AluOpType.add)
            c_new = state.tile([128, HID], F32, tag="c" + ch.name,
                               name="c" + nm)
            nc.gpsimd.tensor_add(c_new[:], ch.fc[:], ig[:])
            ch.c_prev = c_new

        def emit_pw_tc(ch, s):
            nm = f"{ch.name}{s}"
            tc_ = actp.tile([128, HID], F32, tag="tc", name="th" + nm)
            nc.scalar.activation(tc_[:], ch.c_prev[:], TANH)
            h = scr.tile([128, HID], F16, tag="h" + ch.name, name="h" + nm)
            nc.vector.tensor_mul(h[:], ch.sfio[:, 768:1024], tc_[:])
            ch.h = h

        def emit_xpose(ch, s):
            """Transpose h into X (or scratch during warmup); sets srcs."""
            nm = f"{ch.name}{s}"
            if s >= W:
                t_rel = s - W
                lp = t_rel if ch.dirn == "f" else CHUNK - 1 - t_rel
                dst = X4[:, ch.slot0:ch.slot0 + 2, lp, :]
                d0 = X4[:, ch.slot0, lp, :]
                d1 = X4[:, ch.slot0 + 1, lp, :]
            else:
                hs = scr.tile([128, 256], F16, tag="hTs", name="hs" + nm)
                dst = hs[:].rearrange("p (k c) -> p k c", k=2)
                d0 = hs[:, 0:128]
                d1 = hs[:, 128:256]
            hT_ps = tpsum.tile([128, 256], F16, tag="ht", name="hp" + nm)
            for half, first in ((0, True), (1, False)):
                hsl = slice(half * 128, half * 128 + 128)
                nc.tensor.matmul(hT_ps[:, hsl], ch.h[:, hsl], ident[:],
                                 start=first, stop=first,
                                 is_transpose=True,
                                 skip_group_check=not first)
            nc.vector.tensor_copy(dst,
                                  hT_ps[:].rearrange("p (k c) -> p k c", k=2))
            ch.src0 = d0
            ch.src1 = d1

        # ---- phase 2 machinery: final linear per position-group; groups
        # are emitted inside the main loop as soon as their X columns are
        # complete (F chain wrote lp<=p0+glen-1, B chain wrote lp>=p0).
        o_all = ostage.tile([MEL, POS_CORE * B], F16, tag="oall", name="oall")
        o_v = o_all[:].rearrange("p (pp a t b) -> p pp a t b",
                                 pp=2, a=2, t=CHUNK)

        def emit_group(pp, p0, glen):
            n = glen * 128
            kslots = [2 * pp, 2 * pp + 1, 4 + 2 * pp, 5 + 2 * pp]
            ps = gpsum.tile([MEL, 512], F32, tag="g", name=f"op{pp}_{p0}")
            for k, q in enumerate(kslots):
                nc.tensor.matmul(ps[:, 0:n],
                                 lin_w[:, k * MEL:(k + 1) * MEL],
                                 X4[:, q, p0:p0 + glen, :],
                                 start=(k == 0), stop=(k == 3))
            srcv = ps[:, 0:n].rearrange("p (t a b) -> p a t b", t=glen, a=2)
            nc.scalar.activation(o_v[:, pp, :, p0:p0 + glen], srcv, IDENT,
                                 bias=lin_b[:])

        o_dmav = out_d[:].rearrange("p (pp a t) b -> p pp a t b",
                                    pp=2, a=2, t=CHUNK)

        def emit_out_dma_part(pp, t0, t1):
            src = o_v[:, pp, :, t0:t1, :]
            nc.sync.dma_start(o_dmav[:, pp, :, t0:t1, :], src)

        grp_queue = []
        grp_left = [0, 0]
        for pp in range(2):
            p0 = 0
            while p0 < CHUNK:
                glen = min(4, CHUNK - p0)
                rdy = max(4 * (W + p0 + glen - 1) + pp,
                          4 * (W + CHUNK - 1 - p0) + 2 + pp) + 6
                grp_queue.append((rdy, pp, p0, glen))
                grp_left[pp] += 1
                p0 += glen
        # post-loop only, in build order (pp-major, p0 ascending):
        # in-loop injection stretches slots 1:1 with the tail gain; the
        # gates ring is free after the loop and pipelines groups cleanly
        nsl_ = K_STEPS * NCH
        grp_queue = [(nsl_ + k, pp, p0, glen)
                     for k, (rdy, pp, p0, glen) in enumerate(grp_queue)]

        # ---- flat slot-stream schedule with lagged emission ----
        slots = [(s, chains[i]) for s in range(K_STEPS) for i in range(NCH)]
        for ch in chains:
            ch.gates = emit_xe_mms(ch, 0)
        emit_rec_mms(chains[0], 0)

        nslots = len(slots)
        for t, (s, ch) in enumerate(slots):
            if t % NCH == 0:
                load_block(s // XBLK + 1)
            # xe deferred 3 slots: opens the group right before the same
            # chain's rec accumulation below (shortest group lifetime)
            if t >= 3:
                sp, chp = slots[t - 3]
                if sp + 1 < K_STEPS:
                    chp.gates = emit_xe_mms(chp, sp + 1)
            if t + 1 < nslots:
                s2, ch2 = slots[t + 1]
                emit_rec_mms(ch2, s2)
            if t >= 2:
                emit_pw_tc(slots[t - 2][1], slots[t - 2][0])
            emit_pw_sig_a(ch, s)
            emit_pw_sig_b(ch, s)
            if t >= 2:
                emit_xpose(slots[t - 2][1], slots[t - 2][0])
            if grp_queue and grp_queue[0][0] <= t:
                _, pp_, p0_, glen_ = grp_queue.pop(0)
                emit_group(pp_, p0_, glen_)
                grp_left[pp_] -= 1
                if grp_left[pp_] == 3:
                    emit_out_dma_part(pp_, 0, 12)
                elif grp_left[pp_] == 0:
                    emit_out_dma_part(pp_, 12, CHUNK)
        for tt in (nslots - 3, nslots - 2, nslots - 1):
            sp, chp = slots[tt]
            if sp + 1 < K_STEPS:
                chp.gates = emit_xe_mms(chp, sp + 1)
        for tt in (nslots - 2, nslots - 1):
            emit_pw_tc(slots[tt][1], slots[tt][0])
            emit_xpose(slots[tt][1], slots[tt][0])

        # ---- phase 2 drain: groups not ready inside the loop ----
        while grp_queue:
            _, pp_, p0_, glen_ = grp_queue.pop(0)
            emit_group(pp_, p0_, glen_)
            grp_left[pp_] -= 1
            if grp_left[pp_] == 3:
                emit_out_dma_part(pp_, 0, 12)
            elif grp_left[pp_] == 0:
                emit_out_dma_part(pp_, 12, CHUNK)

    nc.compile()
    return nc


def _np_lstm_fallback(exp, inputs):
    def sigmoid(z):
        return 1.0 / (1.0 + np.exp(-z))

    def lstm(xs, wih, whh, bih, bhh):
        Bb, L, E = xs.shape
        pre = np.einsum("ble,ge->blg", xs, wih) + bih + bhh
        h = np.zeros((Bb, HID), np.float32)
        c = np.zeros((Bb, HID), np.float32)
        hs = np.zeros((Bb, L, HID), np.float32)
        for t in range(L):
            gg = pre[:, t] + h @ whh.T
            i, f, g_, o = np.split(gg, 4, axis=-1)
            c = sigmoid(f) * c + sigmoid(i) * np.tanh(g_)
            h = sigmoid(o) * np.tanh(c)
            hs[:, t] = h
        return hs

    out_f = lstm(exp, inputs["wih_f"], inputs["whh_f"], inputs["bih_f"],
                 inputs["bhh_f"])
    out_b = lstm(exp[:, ::-1], inputs["wih_b"], inputs["whh_b"],
                 inputs["bih_b"], inputs["bhh_b"])[:, ::-1]
    out = np.concatenate([out_f, out_b], axis=-1)
    return out @ inputs["lin_w"].T + inputs["lin_b"]


def make_in_maps(expP, expR, inputs):
    perm = _gate_perm()
    gscale = np.ones((4 * HID, 1), np.float32)
    gscale[:HID] = -2.0
    wihT_f = np.ascontiguousarray(
        (inputs["wih_f"].astype(np.float32)[perm] * gscale).T
        ).astype(np.float16)[None]
    wihT_b = np.ascontiguousarray(
        (inputs["wih_b"].astype(np.float32)[perm] * gscale).T
        ).astype(np.float16)[None]
    def _pack(mT, nblk):
        blocks = mT.reshape(nblk, 128, mT.shape[1])
        return np.ascontiguousarray(np.concatenate(list(blocks), axis=1))

    whhT_f = _pack((inputs["whh_f"].astype(np.float32)[perm] * gscale).T
                   .astype(np.float16), 2)
    whhT_b = _pack((inputs["whh_b"].astype(np.float32)[perm] * gscale).T
                   .astype(np.float16), 2)
    linT = _pack(inputs["lin_w"].astype(np.float32).T.astype(np.float16), 4)
    lin_b2 = np.ascontiguousarray(inputs["lin_b"].astype(np.float32)[:, None])
    zeros = np.zeros((128, 256), np.float16)

    expP16 = expP.astype(np.float16)
    expR16 = expR.astype(np.float16)
    in_maps = []
    for j in range(N_CORES):
        xein = np.zeros((EMB, NBLK * XBLK, 512), np.float16)
        ck = [4 * j, 4 * j + 1, 4 * j + 2, 4 * j + 3,
              31 - 4 * j, 30 - 4 * j, 29 - 4 * j, 28 - 4 * j]
        srcs = [expP16] * 4 + [expR16] * 4
        for s in range(K_STEPS):
            for ci, (c, src) in enumerate(zip(ck, srcs)):
                p = c * CHUNK - W + s
                if 0 <= p < L_PAD:
                    xein[:, s, ci * 64:(ci + 1) * 64] = src[:, p].T
        xein = xein.reshape(EMB, NBLK * XBLK * 512)
        in_maps.append({
            "xein": xein,
            "wihT_f": wihT_f, "wihT_b": wihT_b,
            "whhT_f": whhT_f, "whhT_b": whhT_b,
            "linT": linT, "lin_b": lin_b2, "zeros": zeros,
        })
    return in_maps


def kernel(**inputs):
    global _COMPILED
    inputs = {k: np.asarray(v) for k, v in inputs.items()}
    x = inputs["x"].astype(np.int64)
    exp, L = _host_expand(x, inputs["embed"].astype(np.float32),
                          inputs["dp_w"].astype(np.float32),
                          inputs["dp_b"].astype(np.float32))

    bias_mag = max(float(np.abs(inputs[k]).max())
                   for k in ("bih_f", "bhh_f", "bih_b", "bhh_b"))
    if L > L_PAD or bias_mag != 0.0:
        f32in = {k: (v.astype(np.float32) if v.dtype.kind == "f" else v)
                 for k, v in inputs.items()}
        return _np_lstm_fallback(exp, f32in).astype(np.float32)

    expP = np.zeros((B, L_PAD, EMB), np.float32)
    expP[:, :L] = exp
    expR = expP[:, ::-1]

    in_maps = make_in_maps(expP, expR, inputs)

    if _COMPILED is None:
        _COMPILED = _build_kernel()
    nc = _COMPILED

    res = run_bass_kernel_spmd(nc, in_maps, core_ids=list(range(N_CORES)))

    out = np.empty((B, L_PAD, MEL), np.float32)
    for j in range(N_CORES):
        om = res.results[j]["out_mel"]          # [MEL, POS_CORE, B] fp16
        out[:, j * POS_CORE:(j + 1) * POS_CORE] = \
            om.transpose(2, 1, 0).astype(np.float32)
    return np.ascontiguousarray(out[:, :L])


if __name__ == "__main__":
    inputs = dict(np.load("/root/problem/inputs.npz"))
    out = kernel(**inputs)
    ref = np.load("/root/problem/expected.npy")
    diff = np.abs(out - ref)
    print("out", out.shape, "absmax diff", diff.max(),
          "rel", diff.max() / np.abs(ref).max())
